# revision 39
# baseline (speedup 1.0000x reference)
"""Multi-head causal attention (B=2, S=2048, E=1024, H=16, D=64) on 8 TRN2
NeuronCores.

Sharding (data + tensor parallel, Megatron-style):
  core c -> batch b = c // 4, head group g = c % 4 (4 heads, e' = 256 cols).
  Wq/Wk/Wv column-sharded ([256, 1024] slices), Wo row-sharded
  ([1024, 256] slice); each core produces a partial output [2048, 1024]
  (f16) which the host sums per batch group (the Megatron all-reduce) and
  adds bo.

Per-core device kernel (matmul operands fp16, accumulate fp32 in PSUM),
scheduled as one unified slot stream with a deadline-gated filler queue:
  - input DMAs split across BOTH hardware DGE rings (SP + Activation —
    the ACT queue is exp-idle early, but only its first 4 DMAs, before
    semaphore rotation would add reuse-waits in front of the exp stream);
    issue order == arrival order, exp-critical path (wk, xk0 | wq, xq3)
    first.
  - logits pairs (2 heads on PE row-tiles 0-63/64-127) run concurrently;
    attnV (V' ones column -> PSUM row 64 = denominator) trails the exp
    stream by a per-phase lag and crosses (qt, c) phase boundaries, so
    the next phase's logits keep the ACT exp stream fed while the
    previous phase drains and normalizes.
  - projection/O-proj matmuls are pumped one-at-a-time (front-of-queue
    only: max one chain mid-flight on the shared PSUM tag) from a
    deadline-ordered filler queue; per-chain gates stop the pump from
    issuing matmuls whose DMA inputs haven't landed (an in-order PE
    queue would stall); modest per-phase budgets keep PE duty smooth
    enough to stay under the HAM power throttle until the tail.
  - normalize: accs copied to SBUF (frees PSUM), denominator rows
    DMA-transposed to [128, 4]/head for a partition-parallel DVE
    reciprocal (the [1, 512] row form costs ~4us/call, and
    reciprocal_approx_fast NaNs on hardware), transposed back, one
    gpsimd partition-broadcast for both heads, DVE multiply into valsT.
  - O-proj chains register only after the (qt, c=1) normalize that
    writes their valsT window has been issued.
"""
import sys
import os

sys.path.insert(0, "/opt/trn_rl_repo")

import numpy as np
from contextlib import ExitStack

import concourse.bass as bass  # noqa: E402
import concourse.mybir as mybir  # noqa: E402
import concourse.tile as tile  # noqa: E402
from concourse import bacc, bass_utils  # noqa: E402

bass_utils.upload_artifacts = lambda d: f"local:{d}"

B, S, E, H, D = 2, 2048, 1024, 16, 64
NCORES = 8
EL = 256  # e' columns per core (4 heads)
F32 = mybir.dt.float32
F16 = mybir.dt.float16
AF = mybir.ActivationFunctionType

_CACHE = {}

LAG3 = 6   # attnV lag during (qt=3, c=0): waits for xv DMA + vproj
LAG = 2    # attnV lag elsewhere


def _build():
    nc = bacc.Bacc("TRN2", target_bir_lowering=False, debug=False)

    # x tensors host-pre-blocked: [tb, p, k*512 + m]
    xq_d = nc.dram_tensor("xqB", [4, 128, 8 * 512], F16, kind="ExternalInput")
    xk_d = nc.dram_tensor("xkB", [4, 128, 8 * 512], F16, kind="ExternalInput")
    xv_d = nc.dram_tensor("xvB", [4, 128, 8 * 512], F16, kind="ExternalInput")
    wq_d = nc.dram_tensor("wqT", [E, EL], F16, kind="ExternalInput")
    wk_d = nc.dram_tensor("wkT", [E, EL], F16, kind="ExternalInput")
    wv_d = nc.dram_tensor("wvT", [E, EL], F16, kind="ExternalInput")
    wo_d = nc.dram_tensor("woT", [EL, E], F16, kind="ExternalInput")
    bq_d = nc.dram_tensor("bq", [EL], F32, kind="ExternalInput")
    bk_d = nc.dram_tensor("bk", [EL], F32, kind="ExternalInput")
    bv_d = nc.dram_tensor("bv", [EL], F32, kind="ExternalInput")
    vones_d = nc.dram_tensor("vones", [128, 16, 4, 1], F16, kind="ExternalInput")
    mask_d = nc.dram_tensor("masks", [128, 2, 128], F16, kind="ExternalInput")
    out_d = nc.dram_tensor("out", [S, E], F16, kind="ExternalOutput")

    with tile.TileContext(nc) as tc, ExitStack() as ctx:
        cpool = ctx.enter_context(tc.tile_pool(name="const", bufs=1))
        psp = ctx.enter_context(tc.tile_pool(name="psp", bufs=2, space="PSUM"))
        expp = ctx.enter_context(tc.tile_pool(name="expp", bufs=10))
        opool = ctx.enter_context(tc.tile_pool(name="op", bufs=2))
        smp = ctx.enter_context(tc.tile_pool(name="smp", bufs=2))

        xk = cpool.tile([128, 8, S], F16, tag="xk")
        xq = cpool.tile([128, 8, S], F16, tag="xq")
        xv = cpool.tile([128, 8, S], F16, tag="xv")

        def xblock(x_t, x_d, tb, eng=None, halves=False):
            src = x_d.ap()[tb].rearrange("p (k m) -> p k m", k=8)
            if halves:  # split by k so projections chase the DMA per-chunk
                (eng or nc.sync).dma_start(
                    x_t[:, 0:4, tb * 512:(tb + 1) * 512], src[:, 0:4])
                (eng or nc.sync).dma_start(
                    x_t[:, 4:8, tb * 512:(tb + 1) * 512], src[:, 4:8])
            else:
                (eng or nc.sync).dma_start(
                    x_t[:, :, tb * 512:(tb + 1) * 512], src)

        # ---- DMA issue order == arrival order, split across the TWO
        # hardware DGE rings (SP + Activation; ACT is exp-idle until ~20us
        # so its queue is free for input staging). Critical path to first
        # exp: wk+xk0 on SP concurrently with wq+xq3 on ACT.
        wk = cpool.tile([128, 8, EL], F16, tag="wk")
        nc.sync.dma_start(wk[:], wk_d.ap().rearrange("(k p) m -> p k m", p=128))
        xblock(xk, xk_d, 0, halves=True)
        wq = cpool.tile([128, 8, EL], F16, tag="wq")
        nc.scalar.dma_start(wq[:], wq_d.ap().rearrange("(k p) m -> p k m", p=128))
        xblock(xq, xq_d, 3, eng=nc.scalar, halves=True)
        bkt = cpool.tile([128, 2], F32, tag="bkt")
        nc.sync.dma_start(bkt[:], bk_d.ap().rearrange("(c p) -> p c", p=128))
        bqt = cpool.tile([128, 2], F32, tag="bqt")
        nc.sync.dma_start(bqt[:], bq_d.ap().rearrange("(c p) -> p c", p=128))
        mk2 = cpool.tile([128, 2, 128], F16, tag="mk2")
        nc.sync.dma_start(mk2[:], mask_d.ap())
        xblock(xk, xk_d, 1, eng=nc.scalar)
        wv = cpool.tile([128, 8, EL], F16, tag="wv")
        nc.sync.dma_start(wv[:], wv_d.ap().rearrange("(k p) m -> p k m", p=128))
        bvr = cpool.tile([1, EL], F32, tag="bvr")
        nc.sync.dma_start(bvr[:], bv_d.ap().rearrange("(p m) -> p m", p=1))
        bvb = cpool.tile([128, EL], F32, tag="bvb")
        nc.gpsimd.partition_broadcast(bvb[:], bvr[:])
        VP = cpool.tile([128, 16, 4 * 66], F16, tag="VP")  # 66: 4B-aligned
        nc.sync.dma_start(
            VP[:].rearrange("p k (h x) -> p k h x", h=4)[:, :, :, 64:65],
            vones_d.ap())
        xblock(xv, xv_d, 0, eng=nc.scalar)
        xblock(xk, xk_d, 2)
        xblock(xk, xk_d, 3)
        xblock(xv, xv_d, 1)
        xblock(xv, xv_d, 2)
        xblock(xq, xq_d, 2)
        wo = cpool.tile([128, 2, E], F16, tag="wo")
        nc.sync.dma_start(wo[:], wo_d.ap().rearrange("(c p) m -> p c m", p=128))
        xblock(xv, xv_d, 3)
        xblock(xq, xq_d, 1)
        xblock(xq, xq_d, 0)

        KT = cpool.tile([128, 2, S], F16, tag="KT")
        QT = cpool.tile([128, 2, S], F16, tag="QT")
        valsT = cpool.tile([128, 2, S], F16, tag="valsT")

        # ---- PE warmup: the first projections otherwise run at the cold
        # 1.2GHz p-state (full clock needs ~3us of continuous PE busy).
        # Burn dummy matmuls on a memset tile while the first DMAs land.
        warm = cpool.tile([128, 512], F16, tag="warm")
        nc.gpsimd.memset(warm[:], 0.0)
        wps = psp.tile([128, 512], F32, tag="ops", bufs=2, name="warmps")
        for i in range(22):
            nc.tensor.matmul(wps[:], lhsT=warm[:, 0:128], rhs=warm[:, :],
                             start=(i == 0), stop=(i == 21))

        # ---- filler chains: generators yielding after each matmul ----
        def g_kproj(tb, tag="ops", cs=(0, 1)):
            for c in cs:
                ps = psp.tile([128, 512], F32, tag=tag,
                              bufs=2, name=f"kps{tb}_{c}")
                for k in range(8):
                    nc.tensor.matmul(
                        ps[:],
                        lhsT=wk[:, k, c * 128:(c + 1) * 128],
                        rhs=xk[:, k, tb * 512:(tb + 1) * 512],
                        start=(k == 0), stop=(k == 7))
                    yield
                nc.vector.tensor_scalar_add(
                    KT[:, c, tb * 512:(tb + 1) * 512], ps[:], bkt[:, c:c + 1])

        def g_qproj(tt, tag="ops", cs=(0, 1)):
            for c in cs:
                ps = psp.tile([128, 512], F32, tag=tag,
                              bufs=2, name=f"qps{tt}_{c}")
                for k in range(8):
                    nc.tensor.matmul(
                        ps[:],
                        lhsT=wq[:, k, c * 128:(c + 1) * 128],
                        rhs=xq[:, k, tt * 512:(tt + 1) * 512],
                        start=(k == 0), stop=(k == 7))
                    yield
                nc.vector.tensor_scalar_add(
                    QT[:, c, tt * 512:(tt + 1) * 512], ps[:], bqt[:, c:c + 1])

        def g_vproj(t3):
            ps = psp.tile([128, EL], F32, tag="ops", bufs=2, name=f"vps{t3}")
            for k in range(8):
                nc.tensor.matmul(
                    ps[:],
                    lhsT=xv[:, k, t3 * 128:(t3 + 1) * 128],
                    rhs=wv[:, k, :],
                    start=(k == 0), stop=(k == 7))
                yield
            nc.vector.tensor_add(
                VP[:, t3, :].rearrange("p (h x) -> p h x", h=4)[:, :, 0:64],
                ps[:].rearrange("p (h x) -> p h x", h=4),
                bvb[:].rearrange("p (h x) -> p h x", h=4))

        def g_oproj(tp, alt_cast=False):
            # one chain covers tile pair (2*tp, 2*tp+1). alt_cast routes
            # half the PSUM->SBUF casts to the (exp-idle at the tail) ACT
            # engine so the final O-proj pipeline isn't DVE-cast-bound.
            for ti in range(2):
                tt = 2 * tp + ti
                ot = opool.tile([128, 2, 512], F16, tag="ot", name=f"ot{tt}")
                for eo in range(2):
                    ps = psp.tile([128, 512], F32, tag="ops", bufs=2,
                                  name=f"ops{tt}_{eo}")
                    for c in range(2):
                        nc.tensor.matmul(
                            ps[:],
                            lhsT=valsT[:, c, tt * 128:(tt + 1) * 128],
                            rhs=wo[:, c, eo * 512:(eo + 1) * 512],
                            start=(c == 0), stop=(c == 1))
                        yield
                    if alt_cast and eo == 0:
                        nc.scalar.activation(ot[:, eo, :], ps[:], AF.Copy)
                    else:
                        nc.vector.tensor_copy(ot[:, eo, :], ps[:])
                nc.sync.dma_start(
                    out_d.ap()[tt * 128:(tt + 1) * 128, :],
                    ot[:].rearrange("p a b -> p (a b)"))

        chains = {}
        gates = {}
        order = []
        slot = [0]  # global lg_exp counter, for DMA-arrival gating

        def add_chain(name, gen, gate=0):
            chains[name] = gen
            gates[name] = gate
            order.append(name)

        def pump(n, force=False):
            # strictly front-of-queue: at most one chain mid-flight, so the
            # shared "ops" PSUM tag never has two incomplete accumulations.
            # A gated front chain (its DMA inputs not yet landed) stops the
            # pump — issuing it would stall the in-order PE queue.
            done = 0
            while done < n and order:
                name = order[0]
                if not force and gates[name] > slot[0]:
                    return
                try:
                    next(chains[name])
                    done += 1
                except StopIteration:
                    del chains[name]
                    order.pop(0)

        def ensure(name):
            while name in chains:
                pump(1, force=True)

        # pre-exp critical path: K block 0 + Q tile 3 (first lg needs only
        # these); kproj(1..3)/vproj/qproj chase as filler.
        for _ in g_kproj(0, tag="lg"):
            pass
        for _ in g_qproj(3, tag="lg"):
            pass

        # deadline-ordered filler for qt=3 c=0: kproj by lg-tile need,
        # vproj by attnV(lag) need. Gates = earliest slot at which the
        # chain's DMA inputs have landed (est. from the single-ring order).
        add_chain("k1", g_kproj(1))
        add_chain("v0", g_vproj(0))
        add_chain("v1", g_vproj(1))
        add_chain("k2", g_kproj(2), gate=1)
        add_chain("v2", g_vproj(2))
        add_chain("v3", g_vproj(3))
        add_chain("v4", g_vproj(4), gate=2)
        add_chain("v5", g_vproj(5), gate=2)
        add_chain("k3", g_kproj(3), gate=1)
        for t in range(6, 16):
            add_chain(f"v{t}", g_vproj(t),
                      gate=(2 if t < 8 else 4 if t < 12 else 8))

        # filler arriving later (registered at the phase that precedes
        # their deadline): qproj(2) during qt3c1, oproj(12..15)+qproj(1)
        # during qt2, oproj(8..11)+qproj(0) during qt1, oproj(4..7) during
        # qt0, oproj(0..3) at the tail.
        late = {
            (3, 1): [("q2", lambda: g_qproj(2))],
            (2, 0): [("q1", lambda: g_qproj(1))],
            (1, 0): [("q0", lambda: g_qproj(0))],
        }
        # O-proj chains read valsT written by the (qt, c=1) normalize, so
        # they register only once that normalize has been issued.
        onorm = {
            3: [("o6", lambda: g_oproj(6)), ("o7", lambda: g_oproj(7))],
            2: [("o4", lambda: g_oproj(4)), ("o5", lambda: g_oproj(5))],
            1: [("o2", lambda: g_oproj(2)), ("o3", lambda: g_oproj(3))],
            0: [("o0", lambda: g_oproj(0, True)),
                ("o1", lambda: g_oproj(1, True))],
        }
        # per-(qt, c) pump budget (filler matmuls issued after each lg_exp)
        budget = {
            (3, 0): 4, (3, 1): 4,
            (2, 0): 5, (2, 1): 3,
            (1, 0): 6, (1, 1): 3,
            (0, 0): 8, (0, 1): 6,
        }

        accs = {}
        exs = {}

        def lg_exp(qt, c, kt):
            dd = kt * 128 - qt * 512
            s = max(dd, 0)
            lg = psp.tile([128, 2, 512], F32, tag="lg", bufs=2,
                          name=f"lg{qt}_{c}_{kt}")
            for hh in range(2):
                nc.tensor.matmul(
                    lg[:, hh, s:512],
                    lhsT=KT[hh * 64:(hh + 1) * 64, c,
                            kt * 128:(kt + 1) * 128],
                    rhs=QT[hh * 64:(hh + 1) * 64, c,
                           qt * 512 + s:(qt + 1) * 512],
                    start=True, stop=True)
            ex = expp.tile([128, 2, 512], F16, tag="ex",
                           name=f"ex{qt}_{c}_{kt}")
            nc.scalar.activation(ex[:, :, s:512], lg[:, :, s:512], AF.Exp,
                                 scale=0.125)
            if dd >= 0:  # diagonal tile: lower-tri mask on first 128 cols
                nc.vector.tensor_mul(ex[:, :, s:s + 128],
                                     ex[:, :, s:s + 128], mk2[:])
            exs[(qt, c, kt)] = ex

        def attn_v(qt, c, kt):
            nkt = 4 * qt + 4
            if kt == 0:
                for hh in range(2):
                    accs[(qt, c, hh)] = psp.tile(
                        [65, 512], F32, tag="acc", bufs=2,
                        name=f"acc{qt}_{c}_{hh}")
            ex = exs.pop((qt, c, kt))
            s = max(kt * 128 - qt * 512, 0)
            for hh in range(2):
                h = 2 * c + hh
                nc.tensor.matmul(
                    accs[(qt, c, hh)][:, s:512],
                    lhsT=VP[:, kt, h * 66:h * 66 + 65],
                    rhs=ex[:, hh, s:512],
                    start=(kt == 0), stop=(kt == nkt - 1),
                    skip_group_check=True)

        def normalize(qt, c):
            # acc -> SBUF copy (frees PSUM), denominator rows
            # DMA-transposed to [128, 4]/head so the reciprocal runs
            # partition-parallel, transposed back, one gpsimd broadcast
            # for both heads, DVE multiply into valsT.
            sv = smp.tile([65, 2, 512], F32, tag="sv", bufs=2,
                          name=f"sv{qt}_{c}")
            for hh in range(2):
                nc.vector.tensor_copy(sv[:, hh, :], accs[(qt, c, hh)][:])
            lcol = smp.tile([128, 8], F32, tag="lcol", name=f"lc{qt}_{c}")
            for hh in range(2):
                nc.sync.dma_start(
                    lcol[:, hh * 4:(hh + 1) * 4],
                    sv[64:65, hh, :].rearrange("p (a b) -> p a b", a=128))
            rcol = smp.tile([128, 8], F32, tag="rcol", name=f"rc{qt}_{c}")
            nc.vector.reciprocal(rcol[:], lcol[:])
            rrow = smp.tile([1, 2, 512], F32, tag="rrow", bufs=2,
                            name=f"rr{qt}_{c}")
            for hh in range(2):
                nc.sync.dma_start(
                    rrow[0:1, hh, :].rearrange("p (a b) -> p a b", a=128),
                    rcol[:, hh * 4:(hh + 1) * 4])
            bc = smp.tile([64, 2, 512], F32, tag="bc", bufs=2,
                          name=f"bc{qt}_{c}")
            nc.gpsimd.partition_broadcast(
                bc[:].rearrange("p h m -> p (h m)"),
                rrow[:].rearrange("p h m -> p (h m)"))
            for hh in range(2):
                nc.vector.tensor_mul(
                    valsT[hh * 64:(hh + 1) * 64, c,
                          qt * 512:(qt + 1) * 512],
                    sv[0:64, hh, :], bc[:, hh, :])

        # ---- unified slot stream: lg_exp runs ahead; attnV trails by a
        # per-phase lag and crosses (qt, c) boundaries, so the next
        # phase's logits keep the ACT exp stream fed while the previous
        # phase drains + normalizes.
        stream = []
        for qt in range(3, -1, -1):
            for c in range(2):
                for kt in range(4 * qt + 4):
                    stream.append((qt, c, kt))

        import collections
        pending = collections.deque()

        def drain_ready(i):
            while pending:
                qt, c, kt, j = pending[0]
                lag = LAG3 if (qt == 3 and c == 0) else LAG
                if i - j < lag:
                    return
                pending.popleft()
                if qt == 3 and c == 0:
                    ensure(f"v{kt}")
                attn_v(qt, c, kt)
                if kt == 4 * qt + 3:
                    normalize(qt, c)
                    if c == 1:
                        for name, mk in onorm.get(qt, []):
                            add_chain(name, mk())

        for i, (qt, c, kt) in enumerate(stream):
            if kt == 0:
                for name, mk in late.get((qt, c), []):
                    add_chain(name, mk())
                if qt < 3 and c == 0:
                    ensure(f"q{qt}")
            if qt == 3 and c == 0 and kt >= 4 and kt % 4 == 0:
                ensure(f"k{kt // 4}")
            lg_exp(qt, c, kt)
            slot[0] += 1
            pending.append((qt, c, kt, i))
            pump(budget[(qt, c)])
            drain_ready(i)
        drain_ready(10 ** 9)

        # tail: drain remaining filler (incl. the first q-window's O-proj)
        while order:
            pump(1000, force=True)

    nc.compile()
    return nc


def get_nc():
    if "nc" not in _CACHE:
        _CACHE["nc"] = _build()
    return _CACHE["nc"]


def _masks():
    i = np.arange(128)[:, None]
    j = np.arange(128)[None, :]
    m = (i <= j).astype(np.float16)  # within-window causal: keep k <= q
    return np.broadcast_to(m[:, None, :], (128, 2, 128)).copy()


def _xblocks(x):
    # [S, E] f32 -> [4, 128, 8*512] f16: blk[tb, p, k*512+m] = x[tb*512+m, k*128+p]
    xT = np.ascontiguousarray(x.T).astype(np.float16)  # [E, S]
    return np.ascontiguousarray(
        xT.reshape(8, 128, 4, 512).transpose(2, 1, 0, 3).reshape(4, 128, 4096))


def make_in_maps(query, key, value, Wq, bq, Wk, bk, Wv, bv, Wo, bo):
    query = np.asarray(query, np.float32)
    key = np.asarray(key, np.float32)
    value = np.asarray(value, np.float32)
    Wq, Wk, Wv, Wo = (np.asarray(a, np.float32) for a in (Wq, Wk, Wv, Wo))
    bq, bk, bv = (np.asarray(a, np.float32) for a in (bq, bk, bv))
    masks = _masks()
    vones = np.ones((128, 16, 4, 1), np.float16)
    xb = {}
    for b in range(B):
        xb[b] = (_xblocks(query[b]), _xblocks(key[b]), _xblocks(value[b]))
    in_maps = []
    for c in range(NCORES):
        b, g = divmod(c, 4)
        sl = slice(g * EL, (g + 1) * EL)
        in_maps.append({
            "xqB": xb[b][0],
            "xkB": xb[b][1],
            "xvB": xb[b][2],
            "wqT": np.ascontiguousarray(Wq[sl, :].T).astype(np.float16),
            "wkT": np.ascontiguousarray(Wk[sl, :].T).astype(np.float16),
            "wvT": np.ascontiguousarray(Wv[sl, :].T).astype(np.float16),
            "woT": np.ascontiguousarray(Wo[:, sl].T).astype(np.float16),
            "bq": np.ascontiguousarray(bq[sl]),
            "bk": np.ascontiguousarray(bk[sl]),
            "bv": np.ascontiguousarray(bv[sl]),
            "vones": vones,
            "masks": masks,
        })
    return in_maps


def run(inputs, trace=False, tmpdir=None):
    """Run on 8 cores; returns (full_output, BassKernelResults)."""
    nc = get_nc()
    in_maps = make_in_maps(**inputs)
    res = bass_utils.run_bass_kernel_spmd(
        nc, in_maps, list(range(NCORES)), trace=trace, tmpdir=tmpdir)
    bo = np.asarray(inputs["bo"], np.float32)
    out = np.zeros((B, S, E), np.float32)
    for c in range(NCORES):
        out[c // 4] += res.results[c]["out"]
    out += bo[None, None, :]
    return out, res


def kernel(**inputs):
    out, _ = run(inputs)
    return out
